# revision 33
# baseline (speedup 1.0000x reference)
"""DiagLinear (block-diagonal linear + output interleave + bias) on 8 TRN2 cores.

Reference computation (fp32):
    x:   (B=8, S=2048, P*DIN=4096)
    w:   (P=16, DOUT=256, DIN=256)
    b:   (4096,)
    y[b, s, o*P + p] = sum_i x[b, s, p*DIN + i] * w[p, o, i]  + bias[o*P+p]

Sharding: data parallel over the batch dim — core c computes batch c.

The device kernel is purely DMA-bound: x is pre-transposed on the host into
chunk-transposed bf16 layout (partition = feature-in-chunk), so the device
does no transposes at all:

Per-core kernel (xt_c: [128, 16*4096] bf16 -> y_c: [2048, 4096] bf16):
  for each 128-token tile t (16 total):
    1. DMA xt tile [128 feat, 32*128 tok] (1 MiB bf16)
    2. For each psum quarter q (4 blocks): 8 matmuls
         psum[tok, (pp,o)] += xt_chunk.T @ w_chunk   (lhsT = xt, rhs = w)
    3. DVE adds bias and writes the (o,p)-interleaved bf16 output tile to SBUF
    4. DMA y tile [128, 4096] bf16 out

Host layouts:
  xt[f, (t*32 + j)*128 + tok] = x[t*128 + tok, j*128 + f]   (bf16)
  w [i, (2p + c)*256 + o]     = weight[p, o, 128c + i]      (bf16)
  bias_rep[:, p*256 + o]      = bias[o*16 + p]              (fp32, replicated)
y is computed/stored as bf16 and upcast to fp32 on the host.
"""

import contextlib
import ctypes
import sys
import types

import numpy as np

from concourse import bass, mybir, tile
from concourse.bass_utils import run_bass_kernel_spmd


def _install_ntff_shim():
    """Provide antenv.axon_hooks (missing in this image) so trace=True can
    capture NTFF profiles via the axon .so.  Only used when profiling."""
    if "antenv.axon_hooks" in sys.modules:
        return
    so = "/opt/axon/libaxon_pjrt.so"
    try:
        lib = ctypes.CDLL(so)
        lib.axon_start_nrt_profile.argtypes = [
            ctypes.POINTER(ctypes.c_int64),
            ctypes.c_size_t,
        ]
        lib.axon_start_nrt_profile.restype = ctypes.c_int64
        lib.axon_stop_nrt_profile.argtypes = [ctypes.c_char_p]
        lib.axon_stop_nrt_profile.restype = ctypes.c_int64
    except (OSError, AttributeError):
        return

    @contextlib.contextmanager
    def hook(output_dir, device_ids):
        import jax

        jax.devices()
        if device_ids:
            ids = (ctypes.c_int64 * len(device_ids))(*device_ids)
            rc = lib.axon_start_nrt_profile(ids, len(device_ids))
        else:
            rc = lib.axon_start_nrt_profile(None, 0)
        if rc != 0:
            raise RuntimeError(f"axon_start_nrt_profile rc={rc}")
        try:
            yield
        finally:
            n = lib.axon_stop_nrt_profile(str(output_dir).encode())
            print(f"ntff profile: {n} file(s) -> {output_dir}", file=sys.stderr)

    mod = types.ModuleType("antenv.axon_hooks")
    mod.get_axon_ntff_profile_hook = lambda: hook
    mod.set_axon_ntff_profile_hook = lambda h: None
    sys.modules["antenv.axon_hooks"] = mod

P = 16
DIN = 256
DOUT = 256
B = 8
S = 2048
D = P * DIN  # 4096
T_TILE = 128
N_TILES = S // T_TILE  # 16
N_CHUNKS = D // 128  # 32 feature chunks of 128
F32 = mybir.dt.float32
BF16 = mybir.dt.bfloat16


def _split_multi_waits(nc, max_waits=1):
    """This container's walrus build accepts at most one sync-wait per
    instruction; Tile attaches several.  Move the surplus onto dedicated
    single-wait EventSemaphore instructions right before the instruction
    on the same engine (same semantics: the engine is serial)."""
    n_split = 0
    for f in nc.m.functions:
        for bb in f.blocks:
            new_insts = []
            for inst in bb.instructions:
                si = inst.sync_info
                if si is not None and si.on_wait and len(si.on_wait) > max_waits:
                    waits = list(si.on_wait)
                    extra, keep = waits[:-max_waits], waits[-max_waits:]
                    for k, w in enumerate(extra):
                        nop = mybir.InstEventSemaphore(
                            name=f"{inst.name}-wsplit-{k}",
                            engine=inst.engine,
                            sync_info=mybir.SyncInfo(on_wait=[w], on_update=[]),
                        )
                        nc.register_instruction(nop)
                        new_insts.append(nop)
                        n_split += 1
                    inst.sync_info = mybir.SyncInfo(
                        on_wait=keep, on_update=list(si.on_update or [])
                    )
                new_insts.append(inst)
            bb.instructions[:] = new_insts
    return n_split


def build_nc():
    nc = bass.Bass()
    xt_d = nc.declare_dram_parameter("xt", [128, N_TILES * D], BF16, isOutput=False)
    w_d = nc.declare_dram_parameter("w", [128, N_CHUNKS * DOUT], BF16, isOutput=False)
    b_d = nc.declare_dram_parameter("bias_rep", [128, D], BF16, isOutput=False)
    y_d = nc.declare_dram_parameter("y", [S, D], BF16, isOutput=True)

    with tile.TileContext(nc) as tc:
        with (
            tc.tile_pool(name="const", bufs=1) as const_pool,
            tc.tile_pool(name="xt", bufs=1) as pool_xt,
            tc.tile_pool(name="y_sb", bufs=4) as pool_y,
            tc.tile_pool(name="ps_y", bufs=2, space="PSUM") as pool_psy,
        ):
            # PE warmup: the HAM clock gate keeps an idle PE at 1.2 GHz and
            # only releases to 2.4 GHz after ~3.4us of sustained activity.
            # Run junk matmuls on a memset scratch tile while the first DMAs
            # land so the real matmuls start warm (also damps the large
            # run-to-run variance from the free-running HAM window phase).
            scratch = const_pool.tile([128, 512], BF16, tag="scratch")
            nc.gpsimd.memset(scratch[:], 0.0)

            # Startup traffic rides the sync ring in exact dependency order
            # (the SDMA engines round-robin between rings per packet, so a
            # deep x-prefetch on one ring would starve the small critical
            # w/bias transfers on the other).  Half h's matmuls need wh+x0h;
            # ADD h additionally needs bias half h.
            def issue_xt_load(tt):
                xt_t = pool_xt.tile([128, D], BF16, tag=f"x{tt % 6}")
                nc.sync.dma_start(xt_t[:], xt_d[:, tt * D : (tt + 1) * D])
                return xt_t

            x_bufs = {}
            w_tiles = []
            x0_parts = []
            bias_sb = const_pool.tile([128, D], BF16, tag="bias")
            for h in range(2):
                wt_h = const_pool.tile([128, 16 * DOUT], BF16, tag=f"wt{h}")
                w_tiles.append(wt_h)
                x0h = const_pool.tile([128, 2048], BF16, tag=f"x0p{h}")
                x0_parts.append(x0h)
            # piece order = first-consumer order: j0-7 deps, bias q0, j8-15
            # deps, then all of h1
            for h in range(2):
                for g in range(2) if h == 0 else (0,):
                    cols = 2048 if h == 0 else 4096
                    nc.sync.dma_start(
                        w_tiles[h][:, g * 2048 : g * 2048 + cols],
                        w_d[:, h * 4096 + g * 2048 : h * 4096 + g * 2048 + cols],
                    )
                    nc.sync.dma_start(
                        x0_parts[h][:, g * 1024 : g * 1024 + cols // 2],
                        xt_d[:, h * 2048 + g * 1024 : h * 2048 + g * 1024 + cols // 2],
                    )
                    if h == 0 and g == 0:
                        nc.sync.dma_start(
                            bias_sb[:, 0:1024], b_d[:, 0:1024]
                        )
                nc.sync.dma_start(
                    bias_sb[:, h * 2048 + (1024 if h == 0 else 0) : (h + 1) * 2048],
                    b_d[:, h * 2048 + (1024 if h == 0 else 0) : (h + 1) * 2048],
                )
                if h == 0 and N_TILES > 1:
                    x_bufs[1] = issue_xt_load(1)

            def w_ap(j):
                return w_tiles[j // 16][:, (j % 16) * DOUT : (j % 16 + 1) * DOUT]

            def xt_ap(t, xt_tile, j):
                if t == 0:
                    return x0_parts[j // 16][:, (j % 16) * 128 : (j % 16 + 1) * 128]
                return xt_tile[:, j * 128 : (j + 1) * 128]

            # bounded prefetch (depth 4): keeps x rate-matched to compute so
            # the x stream doesn't crowd y stores off the shared SDMA engines
            for tt in range(2, min(4, N_TILES)):
                x_bufs[tt] = issue_xt_load(tt)

            for t in range(N_TILES):
                if t + 4 < N_TILES:
                    x_bufs[t + 4] = issue_xt_load(t + 4)
                y_sb = pool_y.tile([128, D], BF16)
                xt_cur = x_bufs.pop(t, None)
                # tile 0 runs at quarter granularity (4 blocks per psum
                # piece) so the serial DVE ADD chain starts ~3us earlier;
                # steady state uses cheaper half-granularity
                n_pc = 4 if t == 0 else 2
                for hq in range(n_pc):
                    blocks = 16 // n_pc
                    psy = pool_psy.tile([128, blocks * DOUT], F32)
                    if t == 0 and hq == 0:
                        # junk matmuls into regions the real ones overwrite
                        for i in range(18):
                            nc.tensor.matmul(
                                psy[:, 0:512],
                                scratch[:, 0:128],
                                scratch[:],
                                start=True,
                                stop=True,
                            )
                    for pp in range(blocks):
                        p = blocks * hq + pp
                        out = psy[:, pp * DOUT : (pp + 1) * DOUT]
                        for c in (0, 1):
                            j = 2 * p + c
                            nc.tensor.matmul(
                                out,
                                xt_ap(t, xt_cur, j),
                                w_ap(j),
                                start=(c == 0),
                                stop=(c == 1),
                            )
                    # y stays in psum-native (p, o) order — fully contiguous
                    # adds; the host undoes the (o, p) interleave for free
                    # during the bf16 -> fp32 upcast
                    lo = blocks * DOUT * hq
                    nc.vector.tensor_add(
                        y_sb[:, lo : lo + blocks * DOUT],
                        psy[:],
                        bias_sb[:, lo : lo + blocks * DOUT],
                    )
                if t == N_TILES - 1:
                    for hh in range(2):
                        nc.scalar.dma_start(
                            y_d[
                                t * T_TILE : (t + 1) * T_TILE,
                                2048 * hh : 2048 * (hh + 1),
                            ],
                            y_sb[:, 2048 * hh : 2048 * (hh + 1)],
                        )
                else:
                    nc.scalar.dma_start(
                        y_d[t * T_TILE : (t + 1) * T_TILE, :], y_sb[:]
                    )

    _split_multi_waits(nc)
    return nc


def _host_weight(weight, bf16):
    # w_host[i128, (2p + c)*DOUT + o] = weight[p, o, 128c + i128]
    wt = weight.transpose(0, 2, 1).reshape(P, 2, 128, DOUT)  # [p, c, i128, o]
    return np.ascontiguousarray(
        wt.transpose(2, 0, 1, 3).reshape(128, N_CHUNKS * DOUT)
    ).astype(bf16)


def _host_bias(bias, bf16):
    # (p, o) order, replicated over 128 partitions
    bias_po = np.ascontiguousarray(bias.reshape(DOUT, P).T).reshape(-1)
    return np.ascontiguousarray(np.broadcast_to(bias_po, (128, D))).astype(bf16)


def _host_xt(x_c, bf16):
    # xt[f, (t*32 + j)*128 + tok] = x[t*128 + tok, j*128 + f]
    arr = x_c.astype(bf16).reshape(N_TILES, T_TILE, N_CHUNKS, 128)
    return np.ascontiguousarray(arr.transpose(3, 0, 2, 1)).reshape(
        128, N_TILES * D
    )


def kernel(inputs, weight, bias, _trace=False):
    import ml_dtypes

    bf16 = ml_dtypes.bfloat16
    inputs = np.asarray(inputs, dtype=np.float32)
    weight = np.asarray(weight, dtype=np.float32)
    bias = np.asarray(bias, dtype=np.float32)
    assert inputs.shape == (B, S, D)

    if _trace:
        _install_ntff_shim()
    nc = build_nc()
    common = {
        "w": _host_weight(weight, bf16),
        "bias_rep": _host_bias(bias, bf16),
    }
    in_maps = [{"xt": _host_xt(inputs[c], bf16), **common} for c in range(B)]
    res = run_bass_kernel_spmd(nc, in_maps, core_ids=list(range(8)), trace=_trace)
    # device y columns are (p, o)-ordered: col p*256 + o holds y[., o*16 + p]
    out = np.stack(
        [np.asarray(res.results[c]["y"], dtype=np.float32) for c in range(B)],
        axis=0,
    )
    out = np.ascontiguousarray(
        out.reshape(B, S, P, DOUT).transpose(0, 1, 3, 2)
    ).reshape(B, S, D)
    if _trace:
        kernel.last_exec_time_ns = res.exec_time_ns
        kernel.last_results = res
    return out


# revision 34
# speedup vs baseline: 1.0555x; 1.0555x over previous
"""DiagLinear (block-diagonal linear + output interleave + bias) on 8 TRN2 cores.

Reference computation (fp32):
    x:   (B=8, S=2048, P*DIN=4096)
    w:   (P=16, DOUT=256, DIN=256)
    b:   (4096,)
    y[b, s, o*P + p] = sum_i x[b, s, p*DIN + i] * w[p, o, i]  + bias[o*P+p]

Sharding: data parallel over the batch dim — core c computes batch c.

The device kernel is purely DMA-bound: x is pre-transposed on the host into
chunk-transposed bf16 layout (partition = feature-in-chunk), so the device
does no transposes at all:

Per-core kernel (xt_c: [128, 16*4096] bf16 -> y_c: [2048, 4096] bf16):
  for each 128-token tile t (16 total):
    1. DMA xt tile [128 feat, 32*128 tok] (1 MiB bf16)
    2. For each psum quarter q (4 blocks): 8 matmuls
         psum[tok, (pp,o)] += xt_chunk.T @ w_chunk   (lhsT = xt, rhs = w)
    3. DVE adds bias and writes the (o,p)-interleaved bf16 output tile to SBUF
    4. DMA y tile [128, 4096] bf16 out

Host layouts:
  xt[f, (t*32 + j)*128 + tok] = x[t*128 + tok, j*128 + f]   (bf16)
  w [i, (2p + c)*256 + o]     = weight[p, o, 128c + i]      (bf16)
  bias_rep[:, p*256 + o]      = bias[o*16 + p]              (fp32, replicated)
y is computed/stored as bf16 and upcast to fp32 on the host.
"""

import contextlib
import ctypes
import sys
import types

import numpy as np

from concourse import bass, mybir, tile
from concourse.bass_utils import run_bass_kernel_spmd


def _install_ntff_shim():
    """Provide antenv.axon_hooks (missing in this image) so trace=True can
    capture NTFF profiles via the axon .so.  Only used when profiling."""
    if "antenv.axon_hooks" in sys.modules:
        return
    so = "/opt/axon/libaxon_pjrt.so"
    try:
        lib = ctypes.CDLL(so)
        lib.axon_start_nrt_profile.argtypes = [
            ctypes.POINTER(ctypes.c_int64),
            ctypes.c_size_t,
        ]
        lib.axon_start_nrt_profile.restype = ctypes.c_int64
        lib.axon_stop_nrt_profile.argtypes = [ctypes.c_char_p]
        lib.axon_stop_nrt_profile.restype = ctypes.c_int64
    except (OSError, AttributeError):
        return

    @contextlib.contextmanager
    def hook(output_dir, device_ids):
        import jax

        jax.devices()
        if device_ids:
            ids = (ctypes.c_int64 * len(device_ids))(*device_ids)
            rc = lib.axon_start_nrt_profile(ids, len(device_ids))
        else:
            rc = lib.axon_start_nrt_profile(None, 0)
        if rc != 0:
            raise RuntimeError(f"axon_start_nrt_profile rc={rc}")
        try:
            yield
        finally:
            n = lib.axon_stop_nrt_profile(str(output_dir).encode())
            print(f"ntff profile: {n} file(s) -> {output_dir}", file=sys.stderr)

    mod = types.ModuleType("antenv.axon_hooks")
    mod.get_axon_ntff_profile_hook = lambda: hook
    mod.set_axon_ntff_profile_hook = lambda h: None
    sys.modules["antenv.axon_hooks"] = mod

P = 16
DIN = 256
DOUT = 256
B = 8
S = 2048
D = P * DIN  # 4096
T_TILE = 128
N_TILES = S // T_TILE  # 16
N_CHUNKS = D // 128  # 32 feature chunks of 128
F32 = mybir.dt.float32
BF16 = mybir.dt.bfloat16


def _split_multi_waits(nc, max_waits=1):
    """This container's walrus build accepts at most one sync-wait per
    instruction; Tile attaches several.  Move the surplus onto dedicated
    single-wait EventSemaphore instructions right before the instruction
    on the same engine (same semantics: the engine is serial)."""
    n_split = 0
    for f in nc.m.functions:
        for bb in f.blocks:
            new_insts = []
            for inst in bb.instructions:
                si = inst.sync_info
                if si is not None and si.on_wait and len(si.on_wait) > max_waits:
                    waits = list(si.on_wait)
                    extra, keep = waits[:-max_waits], waits[-max_waits:]
                    for k, w in enumerate(extra):
                        nop = mybir.InstEventSemaphore(
                            name=f"{inst.name}-wsplit-{k}",
                            engine=inst.engine,
                            sync_info=mybir.SyncInfo(on_wait=[w], on_update=[]),
                        )
                        nc.register_instruction(nop)
                        new_insts.append(nop)
                        n_split += 1
                    inst.sync_info = mybir.SyncInfo(
                        on_wait=keep, on_update=list(si.on_update or [])
                    )
                new_insts.append(inst)
            bb.instructions[:] = new_insts
    return n_split


def build_nc():
    nc = bass.Bass()
    xt_d = nc.declare_dram_parameter("xt", [128, N_TILES * D], BF16, isOutput=False)
    w_d = nc.declare_dram_parameter("w", [128, N_CHUNKS * DOUT], BF16, isOutput=False)
    b_d = nc.declare_dram_parameter("bias_rep", [128, D], BF16, isOutput=False)
    y_d = nc.declare_dram_parameter("y", [S, D], BF16, isOutput=True)

    with tile.TileContext(nc) as tc:
        with (
            tc.tile_pool(name="const", bufs=1) as const_pool,
            tc.tile_pool(name="xt", bufs=1) as pool_xt,
            tc.tile_pool(name="y_sb", bufs=4) as pool_y,
            tc.tile_pool(name="ps_y", bufs=2, space="PSUM") as pool_psy,
        ):
            # PE warmup: the HAM clock gate keeps an idle PE at 1.2 GHz and
            # only releases to 2.4 GHz after ~3.4us of sustained activity.
            # Run junk matmuls on a memset scratch tile while the first DMAs
            # land so the real matmuls start warm (also damps the large
            # run-to-run variance from the free-running HAM window phase).
            scratch = const_pool.tile([128, 512], BF16, tag="scratch")
            nc.gpsimd.memset(scratch[:], 0.0)

            # Startup traffic rides the sync ring in exact dependency order
            # (the SDMA engines round-robin between rings per packet, so a
            # deep x-prefetch on one ring would starve the small critical
            # w/bias transfers on the other).  Half h's matmuls need wh+x0h;
            # ADD h additionally needs bias half h.
            w_tiles = []
            x0_parts = []
            bias_sb = const_pool.tile([128, D], BF16, tag="bias")
            for h in range(2):
                wt_h = const_pool.tile([128, 16 * DOUT], BF16, tag=f"wt{h}")
                w_tiles.append(wt_h)
                x0h = const_pool.tile([128, 2048], BF16, tag=f"x0p{h}")
                x0_parts.append(x0h)
            # piece order = first-consumer order: j0-7 deps, bias q0, j8-15
            # deps, then all of h1
            for h in range(2):
                for g in range(2) if h == 0 else (0,):
                    cols = 2048 if h == 0 else 4096
                    nc.sync.dma_start(
                        w_tiles[h][:, g * 2048 : g * 2048 + cols],
                        w_d[:, h * 4096 + g * 2048 : h * 4096 + g * 2048 + cols],
                    )
                    nc.sync.dma_start(
                        x0_parts[h][:, g * 1024 : g * 1024 + cols // 2],
                        xt_d[:, h * 2048 + g * 1024 : h * 2048 + g * 1024 + cols // 2],
                    )
                    if h == 0 and g == 0:
                        nc.sync.dma_start(
                            bias_sb[:, 0:1024], b_d[:, 0:1024]
                        )
                nc.sync.dma_start(
                    bias_sb[:, h * 2048 + (1024 if h == 0 else 0) : (h + 1) * 2048],
                    b_d[:, h * 2048 + (1024 if h == 0 else 0) : (h + 1) * 2048],
                )

            def w_ap(j):
                return w_tiles[j // 16][:, (j % 16) * DOUT : (j % 16 + 1) * DOUT]

            def xt_ap(t, xt_tile, j):
                if t == 0:
                    return x0_parts[j // 16][:, (j % 16) * 128 : (j % 16 + 1) * 128]
                return xt_tile[:, j * 128 : (j + 1) * 128]

            # bounded prefetch (depth 4): keeps x rate-matched to compute so
            # the x stream doesn't crowd y stores off the shared SDMA engines
            def issue_xt_load(tt):
                xt_t = pool_xt.tile([128, D], BF16, tag=f"x{tt % 6}")
                nc.sync.dma_start(xt_t[:], xt_d[:, tt * D : (tt + 1) * D])
                return xt_t

            x_bufs = {tt: issue_xt_load(tt) for tt in range(1, min(4, N_TILES))}

            for t in range(N_TILES):
                if t + 4 < N_TILES:
                    x_bufs[t + 4] = issue_xt_load(t + 4)
                y_sb = pool_y.tile([128, D], BF16)
                xt_cur = x_bufs.pop(t, None)
                # tile 0 runs at quarter granularity (4 blocks per psum
                # piece) so the serial DVE ADD chain starts ~3us earlier;
                # steady state uses cheaper half-granularity
                n_pc = 4 if t == 0 else 2
                for hq in range(n_pc):
                    blocks = 16 // n_pc
                    psy = pool_psy.tile([128, blocks * DOUT], F32)
                    if t == 0 and hq == 0:
                        # junk matmuls into regions the real ones overwrite
                        for i in range(18):
                            nc.tensor.matmul(
                                psy[:, 0:512],
                                scratch[:, 0:128],
                                scratch[:],
                                start=True,
                                stop=True,
                            )
                    for pp in range(blocks):
                        p = blocks * hq + pp
                        out = psy[:, pp * DOUT : (pp + 1) * DOUT]
                        for c in (0, 1):
                            j = 2 * p + c
                            nc.tensor.matmul(
                                out,
                                xt_ap(t, xt_cur, j),
                                w_ap(j),
                                start=(c == 0),
                                stop=(c == 1),
                            )
                    # y stays in psum-native (p, o) order — fully contiguous
                    # adds; the host undoes the (o, p) interleave for free
                    # during the bf16 -> fp32 upcast
                    lo = blocks * DOUT * hq
                    nc.vector.tensor_add(
                        y_sb[:, lo : lo + blocks * DOUT],
                        psy[:],
                        bias_sb[:, lo : lo + blocks * DOUT],
                    )
                if t == N_TILES - 1:
                    for hh in range(2):
                        nc.scalar.dma_start(
                            y_d[
                                t * T_TILE : (t + 1) * T_TILE,
                                2048 * hh : 2048 * (hh + 1),
                            ],
                            y_sb[:, 2048 * hh : 2048 * (hh + 1)],
                        )
                else:
                    nc.scalar.dma_start(
                        y_d[t * T_TILE : (t + 1) * T_TILE, :], y_sb[:]
                    )

    _split_multi_waits(nc)
    return nc


def _host_weight(weight, bf16):
    # w_host[i128, (2p + c)*DOUT + o] = weight[p, o, 128c + i128]
    wt = weight.transpose(0, 2, 1).reshape(P, 2, 128, DOUT)  # [p, c, i128, o]
    return np.ascontiguousarray(
        wt.transpose(2, 0, 1, 3).reshape(128, N_CHUNKS * DOUT)
    ).astype(bf16)


def _host_bias(bias, bf16):
    # (p, o) order, replicated over 128 partitions
    bias_po = np.ascontiguousarray(bias.reshape(DOUT, P).T).reshape(-1)
    return np.ascontiguousarray(np.broadcast_to(bias_po, (128, D))).astype(bf16)


def _host_xt(x_c, bf16):
    # xt[f, (t*32 + j)*128 + tok] = x[t*128 + tok, j*128 + f]
    arr = x_c.astype(bf16).reshape(N_TILES, T_TILE, N_CHUNKS, 128)
    return np.ascontiguousarray(arr.transpose(3, 0, 2, 1)).reshape(
        128, N_TILES * D
    )


def kernel(inputs, weight, bias, _trace=False):
    import ml_dtypes

    bf16 = ml_dtypes.bfloat16
    inputs = np.asarray(inputs, dtype=np.float32)
    weight = np.asarray(weight, dtype=np.float32)
    bias = np.asarray(bias, dtype=np.float32)
    assert inputs.shape == (B, S, D)

    if _trace:
        _install_ntff_shim()
    nc = build_nc()
    common = {
        "w": _host_weight(weight, bf16),
        "bias_rep": _host_bias(bias, bf16),
    }
    in_maps = [{"xt": _host_xt(inputs[c], bf16), **common} for c in range(B)]
    res = run_bass_kernel_spmd(nc, in_maps, core_ids=list(range(8)), trace=_trace)
    # device y columns are (p, o)-ordered: col p*256 + o holds y[., o*16 + p]
    out = np.stack(
        [np.asarray(res.results[c]["y"], dtype=np.float32) for c in range(B)],
        axis=0,
    )
    out = np.ascontiguousarray(
        out.reshape(B, S, P, DOUT).transpose(0, 1, 3, 2)
    ).reshape(B, S, D)
    if _trace:
        kernel.last_exec_time_ns = res.exec_time_ns
        kernel.last_results = res
    return out


# revision 37
# speedup vs baseline: 1.0679x; 1.0117x over previous
"""DiagLinear (block-diagonal linear + output interleave + bias) on 8 TRN2 cores.

Reference computation (fp32):
    x:   (B=8, S=2048, P*DIN=4096)
    w:   (P=16, DOUT=256, DIN=256)
    b:   (4096,)
    y[b, s, o*P + p] = sum_i x[b, s, p*DIN + i] * w[p, o, i]  + bias[o*P+p]

Sharding: data parallel over the batch dim — core c computes batch c.

The device kernel is purely DMA-bound: x is pre-transposed on the host into
chunk-transposed bf16 layout (partition = feature-in-chunk), so the device
does no transposes at all:

Per-core kernel (xt_c: [128, 16*4096] bf16 -> y_c: [2048, 4096] bf16):
  for each 128-token tile t (16 total):
    1. DMA xt tile [128 feat, 32*128 tok] (1 MiB bf16)
    2. For each psum quarter q (4 blocks): 8 matmuls
         psum[tok, (pp,o)] += xt_chunk.T @ w_chunk   (lhsT = xt, rhs = w)
    3. DVE adds bias and writes the (o,p)-interleaved bf16 output tile to SBUF
    4. DMA y tile [128, 4096] bf16 out

Host layouts:
  xt[f, (t*32 + j)*128 + tok] = x[t*128 + tok, j*128 + f]   (bf16)
  w [i, (2p + c)*256 + o]     = weight[p, o, 128c + i]      (bf16)
  bias_rep[:, p*256 + o]      = bias[o*16 + p]              (fp32, replicated)
y is computed/stored as bf16 and upcast to fp32 on the host.
"""

import contextlib
import ctypes
import sys
import types

import numpy as np

from concourse import bass, mybir, tile
from concourse.bass_utils import run_bass_kernel_spmd


def _install_ntff_shim():
    """Provide antenv.axon_hooks (missing in this image) so trace=True can
    capture NTFF profiles via the axon .so.  Only used when profiling."""
    if "antenv.axon_hooks" in sys.modules:
        return
    so = "/opt/axon/libaxon_pjrt.so"
    try:
        lib = ctypes.CDLL(so)
        lib.axon_start_nrt_profile.argtypes = [
            ctypes.POINTER(ctypes.c_int64),
            ctypes.c_size_t,
        ]
        lib.axon_start_nrt_profile.restype = ctypes.c_int64
        lib.axon_stop_nrt_profile.argtypes = [ctypes.c_char_p]
        lib.axon_stop_nrt_profile.restype = ctypes.c_int64
    except (OSError, AttributeError):
        return

    @contextlib.contextmanager
    def hook(output_dir, device_ids):
        import jax

        jax.devices()
        if device_ids:
            ids = (ctypes.c_int64 * len(device_ids))(*device_ids)
            rc = lib.axon_start_nrt_profile(ids, len(device_ids))
        else:
            rc = lib.axon_start_nrt_profile(None, 0)
        if rc != 0:
            raise RuntimeError(f"axon_start_nrt_profile rc={rc}")
        try:
            yield
        finally:
            n = lib.axon_stop_nrt_profile(str(output_dir).encode())
            print(f"ntff profile: {n} file(s) -> {output_dir}", file=sys.stderr)

    mod = types.ModuleType("antenv.axon_hooks")
    mod.get_axon_ntff_profile_hook = lambda: hook
    mod.set_axon_ntff_profile_hook = lambda h: None
    sys.modules["antenv.axon_hooks"] = mod

P = 16
DIN = 256
DOUT = 256
B = 8
S = 2048
D = P * DIN  # 4096
T_TILE = 128
N_TILES = S // T_TILE  # 16
N_CHUNKS = D // 128  # 32 feature chunks of 128
F32 = mybir.dt.float32
BF16 = mybir.dt.bfloat16


def _split_multi_waits(nc, max_waits=1):
    """This container's walrus build accepts at most one sync-wait per
    instruction; Tile attaches several.  Move the surplus onto dedicated
    single-wait EventSemaphore instructions right before the instruction
    on the same engine (same semantics: the engine is serial)."""
    n_split = 0
    for f in nc.m.functions:
        for bb in f.blocks:
            new_insts = []
            for inst in bb.instructions:
                si = inst.sync_info
                if si is not None and si.on_wait and len(si.on_wait) > max_waits:
                    waits = list(si.on_wait)
                    extra, keep = waits[:-max_waits], waits[-max_waits:]
                    for k, w in enumerate(extra):
                        nop = mybir.InstEventSemaphore(
                            name=f"{inst.name}-wsplit-{k}",
                            engine=inst.engine,
                            sync_info=mybir.SyncInfo(on_wait=[w], on_update=[]),
                        )
                        nc.register_instruction(nop)
                        new_insts.append(nop)
                        n_split += 1
                    inst.sync_info = mybir.SyncInfo(
                        on_wait=keep, on_update=list(si.on_update or [])
                    )
                new_insts.append(inst)
            bb.instructions[:] = new_insts
    return n_split


def build_nc():
    nc = bass.Bass()
    xt_d = nc.declare_dram_parameter("xt", [128, N_TILES * D], BF16, isOutput=False)
    w_d = nc.declare_dram_parameter("w", [128, N_CHUNKS * DOUT], BF16, isOutput=False)
    b_d = nc.declare_dram_parameter("bias_rep", [128, D], BF16, isOutput=False)
    y_d = nc.declare_dram_parameter("y", [S, D], BF16, isOutput=True)

    with tile.TileContext(nc) as tc:
        with (
            tc.tile_pool(name="const", bufs=1) as const_pool,
            tc.tile_pool(name="xt", bufs=1) as pool_xt,
            tc.tile_pool(name="y_sb", bufs=4) as pool_y,
            tc.tile_pool(name="ps_y", bufs=2, space="PSUM") as pool_psy,
        ):
            # PE warmup: the HAM clock gate keeps an idle PE at 1.2 GHz and
            # only releases to 2.4 GHz after ~3.4us of sustained activity.
            # Run junk matmuls on a memset scratch tile while the first DMAs
            # land so the real matmuls start warm (also damps the large
            # run-to-run variance from the free-running HAM window phase).
            scratch = const_pool.tile([128, 512], BF16, tag="scratch")
            nc.gpsimd.memset(scratch[:], 0.0)

            # Startup traffic rides the sync ring in exact dependency order
            # (the SDMA engines round-robin between rings per packet, so a
            # deep x-prefetch on one ring would starve the small critical
            # w/bias transfers on the other).  Half h's matmuls need wh+x0h;
            # ADD h additionally needs bias half h.
            w_tiles = []
            x0_parts = []
            bias_sb = const_pool.tile([128, D], BF16, tag="bias")
            for h in range(2):
                wt_h = const_pool.tile([128, 16 * DOUT], BF16, tag=f"wt{h}")
                w_tiles.append(wt_h)
                x0h = const_pool.tile([128, 2048], BF16, tag=f"x0p{h}")
                x0_parts.append(x0h)
            # piece order = first-consumer order: j0-7 deps, bias q0, j8-15
            # deps, then all of h1
            for h in range(2):
                for g in range(2) if h == 0 else (0,):
                    cols = 2048 if h == 0 else 4096
                    nc.sync.dma_start(
                        w_tiles[h][:, g * 2048 : g * 2048 + cols],
                        w_d[:, h * 4096 + g * 2048 : h * 4096 + g * 2048 + cols],
                    )
                    nc.sync.dma_start(
                        x0_parts[h][:, g * 1024 : g * 1024 + cols // 2],
                        xt_d[:, h * 2048 + g * 1024 : h * 2048 + g * 1024 + cols // 2],
                    )
                    if h == 0 and g == 0:
                        nc.sync.dma_start(
                            bias_sb[:, 0:1024], b_d[:, 0:1024]
                        )
                nc.sync.dma_start(
                    bias_sb[:, h * 2048 + (1024 if h == 0 else 0) : (h + 1) * 2048],
                    b_d[:, h * 2048 + (1024 if h == 0 else 0) : (h + 1) * 2048],
                )

            def w_ap(j):
                return w_tiles[j // 16][:, (j % 16) * DOUT : (j % 16 + 1) * DOUT]

            def xt_ap(t, xt_tile, j):
                if t == 0:
                    return x0_parts[j // 16][:, (j % 16) * 128 : (j % 16 + 1) * 128]
                return xt_tile[:, j * 128 : (j + 1) * 128]

            # bounded prefetch (depth 4): keeps x rate-matched to compute so
            # the x stream doesn't crowd y stores off the shared SDMA engines
            def issue_xt_load(tt):
                xt_t = pool_xt.tile([128, D], BF16, tag=f"x{tt % 6}")
                nc.sync.dma_start(xt_t[:], xt_d[:, tt * D : (tt + 1) * D])
                return xt_t

            x_bufs = {tt: issue_xt_load(tt) for tt in range(1, min(4, N_TILES))}

            for t in range(N_TILES):
                if t + 4 < N_TILES:
                    x_bufs[t + 4] = issue_xt_load(t + 4)
                y_sb = pool_y.tile([128, D], BF16)
                xt_cur = x_bufs.pop(t, None)
                # tile 0 runs at quarter granularity (4 blocks per psum
                # piece) so the serial DVE ADD chain starts ~3us earlier;
                # steady state uses cheaper half-granularity
                n_pc = 4 if t == 0 else 2
                for hq in range(n_pc):
                    blocks = 16 // n_pc
                    psy = pool_psy.tile([128, blocks * DOUT], F32)
                    if t == 0 and hq == 0:
                        # junk matmuls into regions the real ones overwrite
                        for i in range(18):
                            nc.tensor.matmul(
                                psy[:, 0:512],
                                scratch[:, 0:128],
                                scratch[:],
                                start=True,
                                stop=True,
                            )
                    for pp in range(blocks):
                        p = blocks * hq + pp
                        out = psy[:, pp * DOUT : (pp + 1) * DOUT]
                        for c in (0, 1):
                            j = 2 * p + c
                            nc.tensor.matmul(
                                out,
                                xt_ap(t, xt_cur, j),
                                w_ap(j),
                                start=(c == 0),
                                stop=(c == 1),
                            )
                    # y stays in psum-native (p, o) order — fully contiguous
                    # adds; the host undoes the (o, p) interleave for free
                    # during the bf16 -> fp32 upcast
                    lo = blocks * DOUT * hq
                    nc.vector.tensor_add(
                        y_sb[:, lo : lo + blocks * DOUT],
                        psy[:],
                        bias_sb[:, lo : lo + blocks * DOUT],
                    )
                if t == N_TILES - 1:
                    for hh in range(2):
                        nc.scalar.dma_start(
                            y_d[
                                t * T_TILE : (t + 1) * T_TILE,
                                2048 * hh : 2048 * (hh + 1),
                            ],
                            y_sb[:, 2048 * hh : 2048 * (hh + 1)],
                        )
                else:
                    nc.scalar.dma_start(
                        y_d[t * T_TILE : (t + 1) * T_TILE, :], y_sb[:]
                    )

    _split_multi_waits(nc)
    return nc


def _host_weight(weight, bf16):
    # w_host[i128, (2p + c)*DOUT + o] = weight[p, o, 128c + i128]
    wt = weight.transpose(0, 2, 1).reshape(P, 2, 128, DOUT)  # [p, c, i128, o]
    return np.ascontiguousarray(
        wt.transpose(2, 0, 1, 3).reshape(128, N_CHUNKS * DOUT)
    ).astype(bf16)


def _host_bias(bias, bf16):
    # (p, o) order, replicated over 128 partitions
    bias_po = np.ascontiguousarray(bias.reshape(DOUT, P).T).reshape(-1)
    return np.ascontiguousarray(np.broadcast_to(bias_po, (128, D))).astype(bf16)


def _host_xt(x_c, bf16):
    # xt[f, (t*32 + j)*128 + tok] = x[t*128 + tok, j*128 + f]
    arr = x_c.astype(bf16).reshape(N_TILES, T_TILE, N_CHUNKS, 128)
    return np.ascontiguousarray(arr.transpose(3, 0, 2, 1)).reshape(
        128, N_TILES * D
    )


def kernel(inputs, weight, bias, _trace=False):
    import ml_dtypes

    bf16 = ml_dtypes.bfloat16
    inputs = np.asarray(inputs, dtype=np.float32)
    weight = np.asarray(weight, dtype=np.float32)
    bias = np.asarray(bias, dtype=np.float32)
    assert inputs.shape == (B, S, D)

    if _trace:
        _install_ntff_shim()
    nc = build_nc()
    common = {
        "w": _host_weight(weight, bf16),
        "bias_rep": _host_bias(bias, bf16),
    }
    in_maps = [{"xt": _host_xt(inputs[c], bf16), **common} for c in range(B)]
    res = run_bass_kernel_spmd(nc, in_maps, core_ids=list(range(8)), trace=_trace)
    # device y columns are (p, o)-ordered: col p*256 + o holds y[., o*16 + p]
    out = np.stack(
        [np.asarray(res.results[c]["y"], dtype=np.float32) for c in range(B)],
        axis=0,
    )
    out = np.ascontiguousarray(
        out.reshape(B, S, P, DOUT).transpose(0, 1, 3, 2)
    ).reshape(B, S, D)
    if _trace:
        kernel.last_exec_time_ns = res.exec_time_ns
        kernel.last_results = res
    return out
